# revision 23
# baseline (speedup 1.0000x reference)
"""Trainium2 Bass kernel for nn_DAWN_88124138979393 (moe_routing).

Sharding (8 NeuronCores, SPMD — identical instruction stream per core):
  - Token-parallel LN1 / feature / restore / W_O / LN2 / knowledge stages:
    core c owns tokens [c*512, (c+1)*512) of the flattened [4096, 1024]
    residual stream.
  - Head-parallel causal attention: Q^T/K^T/V are exchanged with AllToAlls
    so core c holds heads {2c, 2c+1} for ALL tokens; the normalized
    attention output returns via two half AllToAlls.  V's AllToAll is
    split into two token-half collectives so PV accumulation can start on
    the first half while the second is still in flight.
  - All matmuls run in bf16 (PSUM accumulation fp32).  LN statistics,
    softmax normalization and the residual adds stay fp32.
  - LN 1/sigma is computed as exp(-0.5*ln(var/D+eps)) so every
    scalar-engine op (Ln/Exp/Square/Copy) lives in one activation table —
    no table reloads.
  - Softmax skips max-subtraction (scores are O(1e-3); exp cannot
    overflow) and defers normalization: PV accumulates unnormalized
    exp-weights, a ones-column appended to V yields Z in the same matmul,
    and 1/Z is applied per-token after transposing back to token-major.
  - Attention scores are computed in key-chunk PAIRS: one [128,512] PSUM
    tile holds two 128-key chunks side by side so a single wide exp
    serves both.  A deep cross-block score/exp lookahead with
    half-V-first PV draining hides the V exchange.
  - Restores are single-pass over all chunks into 8 PSUM banks with u
    built in a preceding scope; weight fams stream in halves/quarters,
    double-buffered, so DMA hides under compute.
"""
import sys

sys.path.insert(0, '/opt/trn_rl_repo')

import numpy as np

import bass_rust as _bass_rust
import concourse.bass as bass
import concourse.mybir as mybir
from concourse import tile
from concourse.bass_utils import run_bass_kernel_spmd

dt = mybir.dt
AF = mybir.ActivationFunctionType
ALU = mybir.AluOpType
BF = dt.bfloat16
F32 = dt.float32

B, S, D, H, R, N, RK = 2, 2048, 1024, 16, 64, 32, 128
DH = D // H          # 64
T = B * S            # 4096
NC = 8               # cores
TC = T // NC         # 512 tokens per core
NT = TC // 128       # 4 token tiles per core
NRC = (N * R) // 128   # 16 chunks in the qk/v pools (2 neurons per chunk)
NKC = (N * RK) // 128  # 32 chunks in the knowledge pool (1 neuron per chunk)
DC = D // 128        # 8 d-chunks
EPS = 1e-5

MAX_WAITS_PER_INST = 1
ES_BUFS = 16         # attention exp-tile ring (cross-block lookahead)
ES_LA = 12           # max pending PV pairs before forced flush

# const blob column offsets (bf16 [128, 768])
CB_IDENT = 0      # [128, 128] identity
CB_SEL64 = 128    # [128, 64]  I64 stacked twice
CB_MASK = 192     # [128, 512] causal mask for a kc-pair (m0|m1)
CB_ONES = 704     # [128, 64]  ones (V-interleave Z column fill)
CB_COLS = 768

W_ORDER = ("wfq", "wfk", "wfv", "wrq", "wrk", "wrv", "wkf", "wkr")


# ---------------------------------------------------------------------------
# Tile tail-drain patch: walrus in this container rejects instructions that
# carry more than one sync-wait command.  Split the kernel-tail drain into
# one drain per proc, and post-split every instruction's waits onto NOPs.
# ---------------------------------------------------------------------------

def _split_drain_and_barrier(self, tick_clock, wait_clock):
    gc = tick_clock.global_clock
    ticks = list(gc)
    procs = [i for i, t in enumerate(ticks) if t > 0]
    for g in range(0, max(len(procs), 1), MAX_WAITS_PER_INST):
        group = procs[g:g + MAX_WAITS_PER_INST]
        sub = _bass_rust.VectorClock()
        for i in group:
            sub.require_at_least(i, ticks[i])
        drain_inst = self.nc.sync.drain()
        wait_clock.add_sem_waits(
            drain_inst.ins, _bass_rust.ScopedClock({None: sub})
        )
    self.nc.all_engine_barrier()
    assert self.sems is not None
    popped = self.nc._tile_sem_poison_stack.pop()
    assert popped is self._sem_poison
    self.nc.clear_and_free_semaphores(list(self.sems.allocated().values()))
    self.nc.all_engine_barrier()


tile.TileContext._drain_and_barrier = _split_drain_and_barrier


def split_waits(nc, max_waits=MAX_WAITS_PER_INST):
    counter = [0]
    for f in nc.m.functions:
        for blk in f.blocks:
            i = 0
            while i < len(blk.instructions):
                inst = blk.instructions[i]
                si = inst.sync_info
                if si is not None and len(si.on_wait) > max_waits:
                    waits = list(si.on_wait)
                    si.on_wait = waits[:max_waits]
                    extra = waits[max_waits:]
                    for g in range(0, len(extra), max_waits):
                        nop = _bass_rust.InstNoOp(
                            name=f"WSPLIT-{counter[0]}", ins=[], outs=[])
                        counter[0] += 1
                        nop.engine = inst.engine
                        nop.sync_info = mybir.SyncInfo(
                            on_wait=extra[g:g + max_waits], on_update=[])
                        nc.register_instruction(nop, overwrite=True)
                        blk.instructions.insert(i, nop)
                        i += 1
                i += 1
    return nc


# ---------------------------------------------------------------------------
# Kernel builder
# ---------------------------------------------------------------------------

def build_kernel(with_bv=False):
    nc = bass.Bass()

    x_sh = nc.declare_dram_parameter("x_sh", [TC, D], F32, isOutput=False)
    FQK = nc.declare_dram_parameter("FQK", [128, DC, N * R], BF, isOutput=False)
    FV = nc.declare_dram_parameter("FV", [128, DC, N * R], BF, isOutput=False)
    FKN = nc.declare_dram_parameter("FKN", [128, DC, N * RK], BF, isOutput=False)
    RQK = nc.declare_dram_parameter("RQK", [128, NRC, D], BF, isOutput=False)
    RV = nc.declare_dram_parameter("RV", [128, NRC, D], BF, isOutput=False)
    RKN = nc.declare_dram_parameter("RKN", [128, NKC, D], BF, isOutput=False)
    WOT = nc.declare_dram_parameter("WOT", [128, DC, D], BF, isOutput=False)
    wts = nc.declare_dram_parameter("wts", [N, len(W_ORDER), TC], BF,
                                    isOutput=False)
    selfeat = nc.declare_dram_parameter("selfeat", [N, NRC, 128], BF,
                                        isOutput=False)
    selkn = nc.declare_dram_parameter("selkn", [N, NKC, 128], BF,
                                      isOutput=False)
    cblob = nc.declare_dram_parameter("cblob", [128, CB_COLS], BF,
                                      isOutput=False)
    oblob = nc.declare_dram_parameter("oblob", [1, 128 + TC], BF,
                                      isOutput=False)
    if with_bv:
        bvb = nc.declare_dram_parameter(
            "bvb", [1, 2 * N * R + N * RK], BF, isOutput=False)

    y_sh = nc.declare_dram_parameter("y_sh", [TC, D], F32, isOutput=True)

    groups = [list(range(NC))]

    with tile.TileContext(nc) as tc:
        with (
            tc.tile_pool(name="const", bufs=1) as cpool,
            tc.tile_pool(name="persist", bufs=1) as pp,
            tc.tile_pool(name="chunk", bufs=4) as ch,
            tc.tile_pool(name="dram", bufs=1, space="DRAM") as dram,
        ):
            # ---- initial DMAs: x first (LN1 is the critical path) -------
            x_t = [pp.tile([128, D], F32, name=f"x_t{i}") for i in range(NT)]
            for i in range(NT):
                nc.sync.dma_start(x_t[i][:], x_sh[i * 128:(i + 1) * 128, :])

            cb = cpool.tile([128, CB_COLS], BF, name="cb")
            nc.sync.dma_start(cb[:], cblob[:])
            ident_t = cb[:, CB_IDENT:CB_IDENT + 128]
            sel64_t = cb[:, CB_SEL64:CB_SEL64 + 64]
            mask_t = cb[:, CB_MASK:CB_MASK + 512]
            ones64_t = cb[:, CB_ONES:CB_ONES + 64]
            ob = cpool.tile([1, 128 + TC], BF, name="ob")
            nc.sync.dma_start(ob[:], oblob[:])
            onesr_t = ob[:, 128:128 + TC]
            eps_t = cpool.tile([128, 1], F32, name="eps_t")
            nc.gpsimd.memset(eps_t[:], EPS)
            wt_all = cpool.tile([N, len(W_ORDER), TC], BF, name="wt_all")
            nc.sync.dma_start(wt_all[:], wts[:])
            wt_t = {nm: wt_all[:, wi, :] for wi, nm in enumerate(W_ORDER)}
            selfeat_t = cpool.tile([N, NRC, 128], BF, name="selfeat_t")
            nc.sync.dma_start(selfeat_t[:], selfeat[:])
            selkn_t = cpool.tile([N, NKC, 128], BF, name="selkn_t")
            nc.sync.dma_start(selkn_t[:], selkn[:])

            if with_bv:
                bvb_t = cpool.tile([1, 2 * N * R + N * RK], BF, name="bvb_t")
                nc.sync.dma_start(bvb_t[:], bvb[:])
                bv_of = {"qk": 0, "v": N * R, "kn": 2 * N * R}

            def ln_rsig(var):
                # 1/sigma = exp(-0.5 * ln(var/D + eps)); Ln and Exp live in
                # the same activation table as Square/Copy -> no reloads.
                lnv = ch.tile([128, 1], F32, tag="lnv")
                nc.scalar.activation(lnv[:], var[:], AF.Ln,
                                     scale=1.0 / D, bias=eps_t[:])
                rsig = ch.tile([128, 1], F32, tag="rsig")
                nc.scalar.activation(rsig[:], lnv[:], AF.Exp, scale=-0.5)
                return rsig

            def layernorm_zT(zT_tile, pool, tag, tiles):
                for i in tiles:
                    mu = ch.tile([128, 1], F32, tag="mu")
                    nc.vector.reduce_sum(mu[:], x_t[i][:],
                                         axis=mybir.AxisListType.X)
                    nc.vector.tensor_scalar_mul(mu[:], mu[:], 1.0 / D)
                    cen = ch.tile([128, D], F32, tag="cen", bufs=2)
                    nc.vector.tensor_scalar(cen[:], x_t[i][:], mu[:],
                                            None, ALU.subtract)
                    sq = ch.tile([128, D], F32, tag="cen", bufs=2)
                    var = ch.tile([128, 1], F32, tag="var")
                    nc.scalar.activation(sq[:], cen[:], AF.Square,
                                         accum_out=var[:])
                    rsig = ln_rsig(var)
                    zb = ch.tile([128, D], BF, tag="zb")
                    nc.vector.tensor_scalar(zb[:], cen[:], rsig[:],
                                            None, ALU.mult)
                    for dc in range(DC):
                        ztp = pool.tile([128, 128], BF, tag=f"ztp{tag}")
                        nc.tensor.transpose(
                            ztp[:], zb[:, dc * 128:(dc + 1) * 128], ident_t)
                        nc.scalar.copy(
                            zT_tile[:, dc, i * 128:(i + 1) * 128], ztp[:])

            zT = pp.tile([128, DC, TC], BF, name="zT", tag="zT")
            with tc.tile_pool(name="psLN", bufs=2, space="PSUM") as psLN:
                layernorm_zT(zT, psLN, "a", range(NT))

            # ======== stage pool A: features + q/k/v restores ===========
            with tc.tile_pool(name="wfA", bufs=1) as wf:

                def feature_half(Fdram, half, bvkey, wh_list, nchunks, psF,
                                 hs, tag, pending):
                    def flush_pending():
                        for h_ps, gs, c in pending:
                            nc.tensor.matmul(h_ps[:], sel64_t, gs[:],
                                             start=(c == 0),
                                             stop=(c == nchunks - 1))
                        pending.clear()

                    for cg in range(2):    # 2 groups of 4 chunks per half
                        q = half * 2 + cg
                        fam = wf.tile([128, DC, 512], BF, tag="Ffam",
                                      bufs=2, name=f"F{tag}{q}")
                        nc.sync.dma_start(
                            fam[:], Fdram[:, :, q * 512:(q + 1) * 512])
                        gtp = [psF.tile([128, TC], F32, tag=f"g{k}",
                                        name=f"g{tag}{half}{cg}_{k}")
                               for k in range(4)]
                        for k in range(4):
                            c = (half * 2 + cg) * 4 + k
                            if with_bv:
                                nc.tensor.matmul(
                                    gtp[k][:],
                                    bvb_t[:, bv_of[bvkey] + c * 128:
                                          bv_of[bvkey] + (c + 1) * 128],
                                    onesr_t, start=True, stop=False)
                        for dc in range(DC):
                            for k in range(4):
                                nc.tensor.matmul(
                                    gtp[k][:],
                                    fam[:, dc, k * 128:(k + 1) * 128],
                                    zT[:, dc, :],
                                    start=(not with_bv and dc == 0),
                                    stop=(dc == DC - 1))
                            if dc == 0:
                                flush_pending()
                        for k in range(4):
                            c = (half * 2 + cg) * 4 + k
                            gcp = ch.tile([128, TC], BF, tag="gcp", bufs=4)
                            nc.scalar.copy(gcp[:], gtp[k][:])
                            for wi, wname in enumerate(wh_list):
                                wb = psF.tile([128, TC], F32, tag="wb",
                                              bufs=2)
                                nc.tensor.matmul(wb[:], selfeat_t[:, c, :],
                                                 wt_t[wname],
                                                 start=True, stop=True)
                                gs = ch.tile([128, TC], BF, tag="gs",
                                             bufs=6)
                                nc.vector.tensor_mul(gs[:], gcp[:], wb[:])
                                pending.append((hs[wi], gs, c))

                def hstack(h_ps, name):
                    t = pp.tile([128, TC], BF, name=name)
                    nc.scalar.copy(t[0:64, :], h_ps[:])
                    nc.scalar.copy(t[64:128, :], h_ps[:])
                    return t

                with tc.tile_pool(name="psF", bufs=1, space="PSUM") as psF:
                    hq_ps = psF.tile([64, TC], F32, tag="hq", name="hq_ps")
                    hk_ps = psF.tile([64, TC], F32, tag="hk", name="hk_ps")
                    pend = []
                    for half in range(2):
                        feature_half(FQK, half, "qk", ["wfq", "wfk"], NRC,
                                     psF, [hq_ps, hk_ps], "qk", pend)
                    for h_ps, gs, c in pend:
                        nc.tensor.matmul(h_ps[:], sel64_t, gs[:],
                                         start=(c == 0), stop=(c == NRC - 1))
                    pend.clear()
                    hq_st = hstack(hq_ps, "hq_st")
                    hk_st = hstack(hk_ps, "hk_st")
                    hv_ps = psF.tile([64, TC], F32, tag="hq", name="hv_ps")
                    for half in range(2):
                        feature_half(FV, half, "v", ["wfv"], NRC, psF,
                                     [hv_ps], "v", pend)
                    for h_ps, gs, c in pend:
                        nc.tensor.matmul(h_ps[:], sel64_t, gs[:],
                                         start=(c == 0), stop=(c == NRC - 1))
                    pend.clear()
                    hv_st = hstack(hv_ps, "hv_st")

                # ---- u chunks + restores -------------------------------
                u_t = [pp.tile([128, TC], BF, name=f"u_t{c}", tag=f"u{c}")
                       for c in range(NRC)]

                def build_u(wname, h_st, u_tiles, nchunks, seln, psU):
                    for c in range(nchunks):
                        wb = psU.tile([128, TC], F32, tag="uwb", bufs=2)
                        nc.tensor.matmul(wb[:], seln[:, c, :], wt_t[wname],
                                         start=True, stop=True)
                        nc.vector.tensor_mul(u_tiles[c][:], h_st[:], wb[:])

                def stream_rfam(Rdram, q, nm):
                    # one 4-chunk quarter of a restore fam, 2-buf rotation
                    fam = wf.tile([128, 4, D], BF, tag="Rfam", bufs=2,
                                  name=nm)
                    nc.sync.dma_start(fam[:],
                                      Rdram[:, q * 4:(q + 1) * 4, :])
                    return fam

                qt_ib = dram.tile([D, TC], BF, name="a2aq_in")
                qt_ob = dram.tile([D, TC], BF, name="a2aq_out")
                kt_ib = dram.tile([D, TC], BF, name="a2ak_in")
                kt_ob = dram.tile([D, TC], BF, name="a2ak_out")
                v_ib = [dram.tile([NC, TC // 2, 128], BF, name=f"a2av_in{h}")
                        for h in range(2)]
                v_ob = [dram.tile([NC, TC // 2, 128], BF,
                                  name=f"a2av_out{h}") for h in range(2)]

                def restore_T(out_name, ib, wname, h_st):
                    # single pass over all 16 chunks into 8 dc-banks
                    with tc.tile_pool(name=f"psU{out_name}", bufs=1,
                                      space="PSUM") as psU:
                        build_u(wname, h_st, u_t, NRC, selfeat_t, psU)
                    with tc.tile_pool(name=f"ps{out_name}", bufs=1,
                                      space="PSUM") as psR:
                        ps = [psR.tile([128, TC], F32, tag=f"rt{k}",
                                       name=f"{out_name}ps{k}")
                              for k in range(DC)]
                        for c in range(NRC):
                            if c % 4 == 0:
                                fam = stream_rfam(RQK, c // 4,
                                                  f"rqk{out_name}{c // 4}")
                            for k in range(DC):
                                nc.tensor.matmul(
                                    ps[k][:],
                                    fam[:, c % 4, k * 128:(k + 1) * 128],
                                    u_t[c][:], start=(c == 0),
                                    stop=(c == NRC - 1))
                        for k in range(DC):
                            ot = ch.tile([128, TC], BF, tag="rT", bufs=4)
                            nc.scalar.copy(ot[:], ps[k][:])
                            nc.sync.dma_start(
                                ib[k * 128:(k + 1) * 128, :], ot[:])

                restore_T("qT", qt_ib, "wrq", hq_st)
                nc.gpsimd.collective_compute(
                    "AllToAll", ALU.bypass, replica_groups=groups,
                    ins=[qt_ib.opt()], outs=[qt_ob.opt()])

                restore_T("kT", kt_ib, "wrk", hk_st)
                nc.gpsimd.collective_compute(
                    "AllToAll", ALU.bypass, replica_groups=groups,
                    ins=[kt_ib.opt()], outs=[kt_ob.opt()])

                # V restore: two token-half passes, each followed by its
                # own half AllToAll so PV can start early.
                with tc.tile_pool(name="psUv", bufs=1, space="PSUM") as psUv:
                    build_u("wrv", hv_st, u_t, NRC, selfeat_t, psUv)
                for half in range(2):
                    with tc.tile_pool(name=f"psV{half}", bufs=1,
                                      space="PSUM") as psV:
                        vps = [psV.tile([128, 512], F32, tag=f"rt{k}",
                                        name=f"vps{half}_{k}")
                               for k in range(4)]
                        for c in range(NRC):
                            if c % 4 == 0:
                                fam = stream_rfam(RV, c // 4,
                                                  f"rv{half}_{c // 4}")
                            for k in range(4):
                                i = half * 2 + k // 2
                                j = k % 2
                                nc.tensor.matmul(
                                    vps[k][:],
                                    u_t[c][:, i * 128:(i + 1) * 128],
                                    fam[:, c % 4, j * 512:(j + 1) * 512],
                                    start=(c == 0), stop=(c == NRC - 1))
                        for i2 in range(2):
                            vsb = ch.tile([128, D], BF, tag="vT", bufs=2)
                            for j in range(2):
                                nc.scalar.copy(
                                    vsb[:, j * 512:(j + 1) * 512],
                                    vps[i2 * 2 + j][:])
                            nc.sync.dma_start(
                                v_ib[half].rearrange("p t e -> t p e")
                                    [i2 * 128:(i2 + 1) * 128],
                                vsb[:].rearrange("q (p e) -> q p e", e=128))
                    nc.gpsimd.collective_compute(
                        "AllToAll", ALU.bypass, replica_groups=groups,
                        ins=[v_ib[half].opt()], outs=[v_ob[half].opt()])

            # ======== stage pool B: attention-era weights ===============
            with tc.tile_pool(name="wfB", bufs=1) as wfB:
                wot_fam = wfB.tile([128, DC, D], BF, tag="WOfam",
                                   name="wot_fam")
                nc.sync.dma_start(wot_fam[:], WOT[:])

                # ---- attention (heads 2c, 2c+1; all tokens) ------------
                qTh = wfB.tile([128, NC, TC], BF, name="qTh")
                kTh = wfB.tile([128, NC, TC], BF, name="kTh")
                nc.sync.dma_start(
                    qTh[:], qt_ob.rearrange("(r p) t -> p r t", p=128))
                nc.sync.dma_start(
                    kTh[:], kt_ob.rearrange("(r p) t -> p r t", p=128))
                qTf = qTh.rearrange("p r t -> p (r t)")
                kTf = kTh.rearrange("p r t -> p (r t)")

                # prefetch the first two FKN eighths; they transfer during
                # the V exchange / early attention
                fkn_pre = []
                for q in range(2):
                    fam = wfB.tile([128, DC, 512], BF, tag="FKfam",
                                   bufs=2, name=f"fknpre{q}")
                    nc.sync.dma_start(
                        fam[:], FKN[:, :, q * 512:(q + 1) * 512])
                    fkn_pre.append(fam)

                # vi_all[p, hp, kb, 0:64] = V for global key chunk kb;
                # kb = r*4 + vh*2 + i  (vh = which V half-collective)
                vi_all = wfB.tile([128, 2, T // 128, 65], BF, name="vi_all")
                vi_v = vi_all[:, :, :, 0:64].rearrange(
                    "p h (r f i) e -> p h r f i e", f=2, i=2)
                for vh in range(2):
                    v_ov = v_ob[vh].rearrange(
                        "r (i p) (h e) -> p r i h e", p=128, e=64)
                    for hp in range(2):
                        for i in range(2):
                            nc.sync.dma_start(
                                vi_v[:, hp, :, vh, i, :],
                                v_ov[:, :, i, hp, :])
                nc.vector.tensor_copy(
                    vi_all[:, :, :, 64].rearrange("p a b -> p (a b)"),
                    ones64_t)

                attnT = wfB.tile([128, T], BF, name="attnT")
                atT = wfB.tile([128, NC, TC], BF, name="atT")

                NQB = S // 256
                blk_order = [(b, qb) for par in range(2)
                             for b in range(B)
                             for qb in range(par, NQB, 2)]

                at_ib = [dram.tile([NC, 128, TC // 2], BF,
                                   name=f"a2aa_in{h}") for h in range(2)]
                at_ob = [dram.tile([NC, 128, TC // 2], BF,
                                   name=f"a2aa_out{h}") for h in range(2)]

                with tc.tile_pool(name="psATT", bufs=1,
                                  space="PSUM") as psT:
                    st_tiles = [psT.tile([128, 512], F32, tag=f"st{k}",
                                         name=f"st{k}") for k in range(2)]
                    ot_tiles = [psT.tile([65, 256], F32, tag=f"ot{k}",
                                         name=f"ot{k}") for k in range(4)]
                    op_tiles = [psT.tile([128, 128], BF, tag=f"op{k}",
                                         name=f"op{k}") for k in range(2)]
                    sti, opi = [0], [0]

                    def next_t(tiles, idx):
                        t = tiles[idx[0] % len(tiles)]
                        idx[0] += 1
                        return t

                    # block records: [b, qb, q0, ots_hp,
                    #                 pv_emitted{hp: n}, finished(bool)]
                    binfo = []
                    pend0, pend1 = [], []   # PV queues by key half class
                    seq = [0]
                    a2a1_done = [False]
                    NEVEN = len(blk_order) // 2

                    def emit_st_pair(hp, kb, b, qb, q0):
                        st = st_tiles[sti[0] % 2]
                        sti[0] += 1
                        for kc in range(2):
                            k0 = b * S + kb * 256 + kc * 128
                            nc.tensor.matmul(
                                st[:, kc * 256:(kc + 1) * 256],
                                kTf[hp * 64:(hp + 1) * 64, k0:k0 + 128],
                                qTf[hp * 64:(hp + 1) * 64, q0:q0 + 256],
                                start=True, stop=True)
                        es = wfB.tile([128, 512], BF, tag="es",
                                      bufs=ES_BUFS)
                        nc.scalar.activation(es[:], st[:], AF.Exp,
                                             scale=0.125)
                        if kb == qb:
                            nc.vector.tensor_mul(es[:], es[:], mask_t)
                        return es

                    def emit_pv(rec):
                        _, bi, es, hp, kb = rec
                        b, qb, q0, ots_hp, emitted, _ = binfo[bi]
                        # finish blocks <= bi-2 first (ot tile reuse)
                        for bj in range(bi - 1):
                            if not binfo[bj][5]:
                                force_finish(bj)
                        npv = 2 * (qb + 1)   # per hp
                        for kc in range(2):
                            k0 = b * S + kb * 256 + kc * 128
                            nc.tensor.matmul(
                                ots_hp[hp][:],
                                vi_all[:, hp, k0 // 128, :],
                                es[:, kc * 256:(kc + 1) * 256],
                                start=(emitted[hp] == 0),
                                stop=(emitted[hp] == npv - 1),
                                skip_group_check=True)
                            emitted[hp] += 1
                        if emitted[0] + emitted[1] == 2 * npv:
                            finish_block(bi)

                    def pop_pv():
                        # strictly prefer half-0 keys: their V collective
                        # lands first, so these PVs never stall the PE on
                        # the second V exchange (emit_pv's force_finish
                        # keeps the ot-tile rotation safe regardless)
                        rec = pend0.pop(0) if pend0 else pend1.pop(0)
                        emit_pv(rec)

                    def force_finish(bi):
                        while not binfo[bi][5]:
                            p0 = [r for r in pend0 if r[1] == bi]
                            p1 = [r for r in pend1 if r[1] == bi]
                            if p0:
                                pend0.remove(p0[0])
                                emit_pv(p0[0])
                            elif p1:
                                pend1.remove(p1[0])
                                emit_pv(p1[0])
                            else:
                                break

                    def finish_block(bi):
                        b, qb, q0, ots_hp, _, _ = binfo[bi]
                        binfo[bi][5] = True
                        aN0 = ch.tile([128, 128], BF, tag="aN", bufs=4)
                        aN1 = ch.tile([128, 128], BF, tag="aN", bufs=4)
                        aNs = [aN0, aN1]
                        for hp in range(2):
                            ots = ch.tile([65, 256], BF, tag="ots", bufs=4)
                            nc.vector.tensor_copy(ots[:], ots_hp[hp][:])
                            for qc in range(2):
                                op = next_t(op_tiles, opi)
                                nc.tensor.transpose(
                                    op[:, 0:65],
                                    ots[:, qc * 128:(qc + 1) * 128],
                                    ident_t[0:65, 0:65])
                                rz = ch.tile([128, 1], F32, tag="rz",
                                             bufs=4)
                                nc.vector.reciprocal(rz[:], op[:, 64:65])
                                nc.vector.tensor_scalar(
                                    aNs[qc][:, hp * 64:(hp + 1) * 64],
                                    op[:, 0:64], rz[:], None, ALU.mult)
                        for qc in range(2):
                            op = next_t(op_tiles, opi)
                            nc.tensor.transpose(op[:], aNs[qc][:], ident_t)
                            ti = q0 + qc * 128
                            nc.scalar.copy(attnT[:, ti:ti + 128], op[:])
                        if (not a2a1_done[0]
                                and all(binfo[j][5]
                                        for j in range(min(NEVEN,
                                                           len(binfo))))
                                and len(binfo) >= NEVEN):
                            # all even blocks finished -> first output a2a
                            a2a1_done[0] = True
                            nc.sync.dma_start(
                                at_ib[0].rearrange("r p t -> p r t"),
                                attnT[:].rearrange(
                                    "p (r h t) -> p r h t",
                                    h=2, t=TC // 2)[:, :, 0, :])
                            nc.gpsimd.collective_compute(
                                "AllToAll", ALU.bypass,
                                replica_groups=groups,
                                ins=[at_ib[0].opt()], outs=[at_ob[0].opt()])
                            nc.sync.dma_start(
                                atT[:].rearrange(
                                    "p r (h t) -> p r h t", h=2)[:, :, 0, :],
                                at_ob[0].rearrange("r p t -> p r t"))

                    for b, qb in blk_order:
                        bi = len(binfo)
                        q0 = b * S + qb * 256
                        ots_hp = [ot_tiles[(2 * bi) % 4],
                                  ot_tiles[(2 * bi + 1) % 4]]
                        binfo.append([b, qb, q0, ots_hp, {0: 0, 1: 0},
                                      False])
                        for kb in range(qb + 1):
                            for hp in range(2):
                                es = emit_st_pair(hp, kb, b, qb, q0)
                                rec = (seq[0], bi, es, hp, kb)
                                seq[0] += 1
                                (pend0 if kb % 2 == 0 else
                                 pend1).append(rec)
                                while len(pend0) + len(pend1) > ES_LA:
                                    pop_pv()
                    while pend0 or pend1:
                        pop_pv()
                    for bi in range(len(binfo)):
                        if not binfo[bi][5]:
                            force_finish(bi)

                nc.sync.dma_start(
                    at_ib[1].rearrange("r p t -> p r t"),
                    attnT[:].rearrange("p (r h t) -> p r h t",
                                       h=2, t=TC // 2)[:, :, 1, :])
                nc.gpsimd.collective_compute(
                    "AllToAll", ALU.bypass, replica_groups=groups,
                    ins=[at_ib[1].opt()], outs=[at_ob[1].opt()])

                # ---- W_O + residual + LN2 + knowledge features ---------
                z2T = pp.tile([128, DC, TC], BF, name="z2T", tag="zT")
                h2_sb = pp.tile([128, TC], BF, name="h2_sb")

                def wo_ln2_half(hf):
                    with tc.tile_pool(name=f"psWO{hf}", bufs=1,
                                      space="PSUM") as psW:
                        aops = [psW.tile([128, 512], F32, tag=f"ao{k}",
                                         name=f"ao{hf}_{k}")
                                for k in range(4)]
                        for dc in range(DC):
                            for i2 in range(2):
                                i = hf * 2 + i2
                                for j in range(2):
                                    nc.tensor.matmul(
                                        aops[i2 * 2 + j][:],
                                        atT[:, dc, i * 128:(i + 1) * 128],
                                        wot_fam[:, dc,
                                                j * 512:(j + 1) * 512],
                                        start=(dc == 0),
                                        stop=(dc == DC - 1))
                        for i2 in range(2):
                            i = hf * 2 + i2
                            for j in range(2):
                                sl = slice(j * 512, (j + 1) * 512)
                                nc.vector.tensor_add(
                                    x_t[i][:, sl], x_t[i][:, sl],
                                    aops[i2 * 2 + j][:])
                    with tc.tile_pool(name=f"psLN2{hf}", bufs=2,
                                      space="PSUM") as psLN2:
                        layernorm_zT(z2T, psLN2, f"b{hf}",
                                     [hf * 2, hf * 2 + 1])

                def kfeat_half(hf):
                    # knowledge features for tokens [hf*256, (hf+1)*256)
                    cs = slice(hf * 256, (hf + 1) * 256)
                    pend2 = []
                    with tc.tile_pool(name=f"psKF{hf}", bufs=1,
                                      space="PSUM") as psK2:
                        h2_ps = psK2.tile([128, 256], F32, tag="h2",
                                          name=f"h2_ps{hf}")

                        def flush_pend2():
                            for gs, c in pend2:
                                nc.tensor.matmul(h2_ps[:], ident_t, gs[:],
                                                 start=(c == 0),
                                                 stop=(c == NKC - 1))
                            pend2.clear()

                        for e in range(8):
                            if hf == 0 and e < 2:
                                fam = fkn_pre[e]
                            else:
                                fam = wfB.tile([128, DC, 512], BF,
                                               tag="FKfam", bufs=2,
                                               name=f"fkn{hf}_{e}")
                                nc.sync.dma_start(
                                    fam[:],
                                    FKN[:, :, e * 512:(e + 1) * 512])
                            gtp = [psK2.tile([128, 256], F32,
                                             tag=f"g{k}",
                                             name=f"g2{hf}{e}_{k}")
                                   for k in range(4)]
                            for k in range(4):
                                c = e * 4 + k
                                if with_bv:
                                    nc.tensor.matmul(
                                        gtp[k][:],
                                        bvb_t[:, bv_of["kn"] + c * 128:
                                              bv_of["kn"] + (c + 1) * 128],
                                        onesr_t[:, cs],
                                        start=True, stop=False)
                            for dc in range(DC):
                                for k in range(4):
                                    nc.tensor.matmul(
                                        gtp[k][:],
                                        fam[:, dc, k * 128:(k + 1) * 128],
                                        z2T[:, dc, cs],
                                        start=(not with_bv and dc == 0),
                                        stop=(dc == DC - 1))
                                if dc == 0:
                                    flush_pend2()
                            for k in range(4):
                                c = e * 4 + k
                                gcp = ch.tile([128, 256], BF,
                                              tag="gcp2", bufs=4)
                                nc.scalar.copy(gcp[:], gtp[k][:])
                                wb = psK2.tile([128, 256], F32,
                                               tag="wb2", bufs=2)
                                nc.tensor.matmul(
                                    wb[:], selkn_t[:, c, :],
                                    wt_t["wkf"][:, cs],
                                    start=True, stop=True)
                                gs = ch.tile([128, 256], BF,
                                             tag="gs2", bufs=6)
                                nc.vector.tensor_mul(gs[:], gcp[:],
                                                     wb[:])
                                pend2.append((gs, c))
                        flush_pend2()
                        nc.scalar.copy(h2_sb[:, cs], h2_ps[:])

                wo_ln2_half(0)
                kfeat_half(0)
                # second output-half load (kept after kfeat(h0)'s fkn DMAs
                # so its a2a#2 wait doesn't head-of-line-block them on SP)
                nc.sync.dma_start(
                    atT[:].rearrange("p r (h t) -> p r h t",
                                     h=2)[:, :, 1, :],
                    at_ob[1].rearrange("r p t -> p r t"))
                wo_ln2_half(1)
                kfeat_half(1)

            # ======== stage pool C: knowledge restore ===================
            with tc.tile_pool(name="wfC", bufs=1) as wfC:
                u2_b = wfC.tile([128, DC, TC], BF, name="u2_b")
                u2_c = wfC.tile([128, DC, TC], BF, name="u2_c")
                u2_t = [pp.tile([128, TC], BF, name=f"u2_t{c}",
                                tag=f"u{c}") for c in range(NRC)]
                u2_t += [u2_b[:, k, :] for k in range(DC)]
                u2_t += [u2_c[:, k, :] for k in range(DC)]

                with tc.tile_pool(name="psU2", bufs=1, space="PSUM") as psU2:
                    for c in range(NKC):
                        wb = psU2.tile([128, TC], F32, tag="uwb", bufs=2)
                        nc.tensor.matmul(wb[:], selkn_t[:, c, :],
                                         wt_t["wkr"], start=True, stop=True)
                        nc.vector.tensor_mul(u2_t[c][:], h2_sb[:], wb[:])
                with tc.tile_pool(name="psKR", bufs=1, space="PSUM") as psKR:
                    kps = [psKR.tile([128, 512], F32, tag=f"kp{k}",
                                     name=f"kp{k}") for k in range(8)]
                    for c in range(NKC):
                        if c % 4 == 0:
                            fam = wfC.tile([128, 4, D], BF, tag="RKfam",
                                           bufs=2, name=f"rkn{c // 4}")
                            nc.sync.dma_start(
                                fam[:], RKN[:, c:c + 4, :])
                        for ti in range(4):
                            for j in range(2):
                                nc.tensor.matmul(
                                    kps[ti * 2 + j][:],
                                    u2_t[c][:, ti * 128:(ti + 1) * 128],
                                    fam[:, c % 4, j * 512:(j + 1) * 512],
                                    start=(c == 0), stop=(c == NKC - 1))
                    for ti in range(4):
                        for j in range(2):
                            sl = slice(j * 512, (j + 1) * 512)
                            nc.vector.tensor_add(
                                x_t[ti][:, sl], x_t[ti][:, sl],
                                kps[ti * 2 + j][:])
                        nc.sync.dma_start(y_sh[ti * 128:(ti + 1) * 128, :],
                                          x_t[ti][:])

    split_waits(nc)
    return nc


# ---------------------------------------------------------------------------
# Host side
# ---------------------------------------------------------------------------

_NC_CACHE = {}


def _get_nc(with_bv=False):
    key = ("nc", with_bv)
    if key not in _NC_CACHE:
        _NC_CACHE[key] = build_kernel(with_bv)
    return _NC_CACHE[key]


def _bf16(a):
    import ml_dtypes
    return np.ascontiguousarray(
        np.asarray(a, dtype=np.float32)).astype(ml_dtypes.bfloat16)


def _chunk_major(a, nchunks):
    # [nchunks*128, M] -> [128, nchunks, M]
    M = a.shape[1]
    return np.ascontiguousarray(
        a.reshape(nchunks, 128, M).transpose(1, 0, 2))


def _selfeat():
    m = np.zeros((N, NRC, 128), np.float32)
    for c in range(NRC):
        m[2 * c, c, 0:64] = 1.0
        m[2 * c + 1, c, 64:128] = 1.0
    return m


def _selkn():
    m = np.zeros((N, NKC, 128), np.float32)
    for c in range(NKC):
        m[c, c, :] = 1.0
    return m


def prepare_inputs(x, f_qk, f_v, r_qk, r_v, f_know, r_know, W_O,
                   gamma1, beta1, gamma2, beta2,
                   w_fq, w_fk, w_fv, w_rq, w_rk, w_rv, w_know_f, w_know_r):
    x = np.asarray(x, np.float32).reshape(T, D)
    gamma1 = np.asarray(gamma1, np.float32)
    beta1 = np.asarray(beta1, np.float32)
    gamma2 = np.asarray(gamma2, np.float32)
    beta2 = np.asarray(beta2, np.float32)

    FQKh = (np.asarray(f_qk, np.float32) * gamma1[None, :, None]) \
        .transpose(1, 0, 2).reshape(D, N * R)
    FVh = (np.asarray(f_v, np.float32) * gamma1[None, :, None]) \
        .transpose(1, 0, 2).reshape(D, N * R)
    FKNh = (np.asarray(f_know, np.float32) * gamma2[None, :, None]) \
        .transpose(1, 0, 2).reshape(D, N * RK)
    RQKh = np.asarray(r_qk, np.float32).reshape(N * R, D)
    RVh = np.asarray(r_v, np.float32).reshape(N * R, D)
    RKNh = np.asarray(r_know, np.float32).reshape(N * RK, D)
    WOTh = np.ascontiguousarray(np.asarray(W_O, np.float32).T)

    with_bv = bool(np.any(beta1 != 0) or np.any(beta2 != 0))

    qi = np.arange(256)[None, :]
    ki = np.arange(128)[:, None]
    cblob = np.zeros((128, CB_COLS), np.float32)
    cblob[:, CB_IDENT:CB_IDENT + 128] = np.eye(128)
    cblob[0:64, CB_SEL64:CB_SEL64 + 64] = np.eye(64)
    cblob[64:128, CB_SEL64:CB_SEL64 + 64] = np.eye(64)
    cblob[:, CB_MASK:CB_MASK + 256] = (qi >= ki)
    cblob[:, CB_MASK + 256:CB_MASK + 512] = (qi >= ki + 128)
    cblob[:, CB_ONES:CB_ONES + 64] = 1.0
    oblob = np.ones((1, 128 + TC), np.float32)

    shared = {
        "FQK": _bf16(_chunk_major(FQKh, DC)),
        "FV": _bf16(_chunk_major(FVh, DC)),
        "FKN": _bf16(_chunk_major(FKNh, DC)),
        "RQK": _bf16(_chunk_major(RQKh, NRC)),
        "RV": _bf16(_chunk_major(RVh, NRC)),
        "RKN": _bf16(_chunk_major(RKNh, NKC)),
        "WOT": _bf16(_chunk_major(WOTh, DC)),
        "selfeat": _bf16(_selfeat()),
        "selkn": _bf16(_selkn()),
        "cblob": _bf16(cblob),
        "oblob": _bf16(oblob),
    }
    if with_bv:
        shared["bvb"] = _bf16(np.concatenate(
            [beta1 @ FQKh, beta1 @ FVh, beta2 @ FKNh])[None, :])

    wmap = {"wfq": w_fq, "wfk": w_fk, "wfv": w_fv, "wrq": w_rq,
            "wrk": w_rk, "wrv": w_rv, "wkf": w_know_f, "wkr": w_know_r}
    in_maps = []
    for c in range(NC):
        m = dict(shared)
        m["x_sh"] = np.ascontiguousarray(x[c * TC:(c + 1) * TC])
        wt = np.stack([
            np.asarray(wmap[nm], np.float32).reshape(T, N)
            [c * TC:(c + 1) * TC].T
            for nm in W_ORDER], axis=1)   # [N, 8, TC]
        m["wts"] = _bf16(wt)
        in_maps.append(m)
    return in_maps, with_bv


def assemble_output(results):
    out = np.empty((T, D), np.float32)
    for c in range(NC):
        out[c * TC:(c + 1) * TC] = results[c]["y_sh"]
    return out.reshape(B, S, D)


def kernel(**inputs):
    in_maps, with_bv = prepare_inputs(**inputs)
    nc = _get_nc(with_bv)
    res = run_bass_kernel_spmd(nc, in_maps, list(range(NC)))
    return assemble_output(res.results)


if __name__ == "__main__":
    build_kernel()
    print("kernel built OK")
